# revision 5
# baseline (speedup 1.0000x reference)
"""Cross-attention kernel for 8 Trainium2 NeuronCores.

Problem: out = softmax((x@Wq)(ctx@Wk)^T * dh^-0.5) @ (ctx@Wv) @ Wo + bo
  x [16, 4096, 320], ctx [16, 77, 768], H=8 heads x DH=40.

Sharding: data-parallel over batch (2 per core), SPMD one NEFF.

Per-core layout ("^T domain": features on partitions, tokens on free dim):
  x^T tiles  <- PE transpose of x
  q^T        <- Wq_pad.T @ x^T      (head pairs 64-aligned: rows 0:40 / 64:104)
  S_h        <- k^T_h.T @ q^T_h     [77 ctx-tok, 512 q-tok] per head
  P_h        <- exp(S_h * scale)    (ACT, fp32r out)
  Zs         <- accumulated selector-column matmuls  [8, 512] (row h = sum_j P_h)
  Rs         <- exp(-ln(Zs))        (ACT; 1/Z without the banned reciprocal)
  Rb         <- E_pair.T @ Rs       broadcast R rows to head-pair layout
  O_pair     <- v_h.T @ P_h (col-tiled pairs), copy*Rb -> SBUF (normalized)
  out        <- O_pair.T @ Wo_pad (+ bo) -> [tokens, 320] -> DMA out

All matmuls run float32r (~1.5e-4 rel err, 1 cycle/row at N>=256).
"""

import numpy as np

H, DH = 8, 40
SCALE = DH ** -0.5
B, N, M = 16, 4096, 77
QD, CD, ID = 320, 768, H * DH
N_CORES = 8
B_LOC = B // N_CORES
CHUNK = 512
NCHUNK = N // CHUNK
NPAIR = H // 2  # head pairs, 64-aligned rows {0:40, 64:104}

_cache = {}


def _legalize_sync_waits(nc, mybir):
    """This walrus build allows 1 sync-wait command per instruction (2 for
    EventSemaphore). Spill extra waits onto same-engine NoOps placed just
    before; per-engine program order makes that equivalent."""
    n = 0
    f = nc.m.functions[0]
    for blk in f.blocks:
        out = []
        changed = False
        for inst in blk.instructions:
            si = inst.sync_info
            waits = list(si.on_wait) if si is not None and si.on_wait else []
            cap = 2 if isinstance(inst, mybir.InstEventSemaphore) else 1
            if len(waits) > cap:
                keep, spill = waits[-cap:], waits[:-cap]
                for w in spill:
                    n += 1
                    nop = mybir.InstNoOp(name=f"I-waitfix-{n}", ins=[], outs=[],
                                         engine=inst.engine)
                    nop.sync_info = mybir.SyncInfo(on_wait=[w], on_update=[])
                    out.append(nop)
                inst.sync_info = mybir.SyncInfo(
                    on_wait=keep,
                    on_update=list(si.on_update) if si.on_update else [])
                changed = True
            out.append(inst)
        if changed:
            blk.instructions = out
    return n


def _aux_arrays(Wq, Wk, Wv, Wo, bo):
    """Host-side constant prep: pad head pairs to 64-aligned rows."""
    def pad_pairs_cols(W):
        # W [c, 320] -> [c, 4, 128]: cols 0:40 = head 2p, 64:104 = head 2p+1
        c = W.shape[0]
        out = np.zeros((c, NPAIR, 128), dtype=np.float32)
        for p in range(NPAIR):
            out[:, p, 0:40] = W[:, 80 * p: 80 * p + 40]
            out[:, p, 64:104] = W[:, 80 * p + 40: 80 * p + 80]
        return out.reshape(c, NPAIR * 128)

    wq_pad = pad_pairs_cols(Wq)                      # [320, 512]
    wk_pad = pad_pairs_cols(Wk)                      # [768, 512]
    # Wo rows padded to pair layout: [4, 128, 320]
    wo_pad = np.zeros((NPAIR, 128, QD), dtype=np.float32)
    for p in range(NPAIR):
        wo_pad[p, 0:40] = Wo[80 * p: 80 * p + 40]
        wo_pad[p, 64:104] = Wo[80 * p + 40: 80 * p + 80]
    # E: [8, 4*128] broadcast map R_h -> pair rows
    e_mat = np.zeros((H, NPAIR * 128), dtype=np.float32)
    for p in range(NPAIR):
        e_mat[2 * p, 128 * p: 128 * p + 40] = 1.0
        e_mat[2 * p + 1, 128 * p + 64: 128 * p + 104] = 1.0
    # selector columns for Zs accumulation: [77, 8], col h = ones
    zcol = np.zeros((M, H, H), dtype=np.float32)
    for h in range(H):
        zcol[:, h, h] = 1.0
    zcol = zcol.reshape(M, H * H)
    ident = np.eye(128, dtype=np.float32)
    return {
        "aux_wq": wq_pad, "aux_wk": wk_pad, "aux_wo": wo_pad.reshape(NPAIR * 128, QD),
        "aux_e": e_mat, "aux_z": zcol, "aux_id": ident,
        "aux_bo": bo.reshape(1, QD).astype(np.float32),
    }


def _build(nc, with_bias):
    import concourse.mybir as mybir
    from concourse.tile import TileContext

    R = mybir.dt.float32r
    F = mybir.dt.float32
    AF = mybir.ActivationFunctionType

    x_d = nc.dram_tensor("x", [B_LOC, N, QD], F, kind="ExternalInput")
    c_d = nc.dram_tensor("context", [B_LOC, M, CD], F, kind="ExternalInput")
    wq_d = nc.dram_tensor("aux_wq", [QD, 512], F, kind="ExternalInput")
    wk_d = nc.dram_tensor("aux_wk", [CD, 512], F, kind="ExternalInput")
    wv_d = nc.dram_tensor("Wv", [CD, ID], F, kind="ExternalInput")
    wo_d = nc.dram_tensor("aux_wo", [512, QD], F, kind="ExternalInput")
    e_d = nc.dram_tensor("aux_e", [H, 512], F, kind="ExternalInput")
    z_d = nc.dram_tensor("aux_z", [M, H * H], F, kind="ExternalInput")
    id_d = nc.dram_tensor("aux_id", [128, 128], F, kind="ExternalInput")
    bo_d = nc.dram_tensor("aux_bo", [1, QD], F, kind="ExternalInput")
    out_d = nc.dram_tensor("out", [B_LOC, N, QD], F, kind="ExternalOutput")

    CKT = [(0, 128), (128, 128), (256, 64)]            # QD k-tiles
    CKT6 = [(128 * i, 128) for i in range(6)]          # CD k-tiles

    with TileContext(nc) as tc:
        with tc.tile_pool(name="wpool", bufs=1) as wp, \
             tc.tile_pool(name="bpool", bufs=2) as bp, \
             tc.tile_pool(name="cpool", bufs=2) as cp, \
             tc.tile_pool(name="epool", bufs=10) as ep, \
             tc.tile_pool(name="ps", bufs=2, space="PSUM") as ps:

            # ---- per-core constants ----
            wq_r = []
            for kt, (o, w) in enumerate(CKT):
                t = wp.tile([w, 512], R, name=f"wq{kt}", tag=f"wq{kt}")
                nc.sync.dma_start(t[:], wq_d[o:o + w, :].bitcast(R))
                wq_r.append(t)
            wk_r = []
            wv_r = []
            for kt, (o, w) in enumerate(CKT6):
                t = wp.tile([w, 512], R, name=f"wk{kt}", tag=f"wk{kt}")
                nc.sync.dma_start(t[:], wk_d[o:o + w, :].bitcast(R))
                wk_r.append(t)
                t2 = wp.tile([w, ID], R, name=f"wv{kt}", tag=f"wv{kt}")
                nc.sync.dma_start(t2[:], wv_d[o:o + w, :].bitcast(R))
                wv_r.append(t2)
            wo_r = []
            for p in range(NPAIR):
                t = wp.tile([128, QD], R, name=f"wo{p}", tag=f"wo{p}")
                nc.sync.dma_start(t[:], wo_d[128 * p:128 * p + 128, :].bitcast(R))
                wo_r.append(t)
            e_r = wp.tile([H, 512], R, name="e_r", tag="e_r")
            nc.sync.dma_start(e_r[:], e_d[:].bitcast(R))
            z_r = wp.tile([M, H * H], R, name="z_r", tag="z_r")
            nc.sync.dma_start(z_r[:], z_d[:].bitcast(R))
            id_r = wp.tile([128, 128], R, name="id_r", tag="id_r")
            nc.sync.dma_start(id_r[:], id_d[:].bitcast(R))
            id_f = wp.tile([128, 128], F, name="id_f", tag="id_f")
            nc.sync.dma_start(id_f[:], id_d[:])
            if with_bias:
                bo_r = wp.tile([1, QD], R, name="bo_r", tag="bo_r")
                nc.sync.dma_start(bo_r[:], bo_d[:].bitcast(R))
                ones_r = wp.tile([1, 128], R, name="ones_r", tag="ones_r")
                nc.vector.memset(ones_r[:].bitcast(F), 1.0)

            for b in range(B_LOC):
                # ---- per-batch: ctx^T, k^T pairs, v ----
                ctx_f = bp.tile([M, CD], F, name="ctx_f", tag="ctx")
                nc.sync.dma_start(ctx_f[:], c_d[b])
                ctxT_r = []
                for s, (o, w) in enumerate(CKT6):
                    pt = ps.tile([w, M], F, name=f"ctT{s}", tag="misc")
                    nc.tensor.transpose(pt[:], ctx_f[:, o:o + w], id_f[:M, :M])
                    st = bp.tile([w, 80], R, name=f"ctxT{s}", tag=f"ctxT{s}")
                    nc.vector.memset(st[:, M:80].bitcast(F), 0.0)
                    nc.vector.tensor_copy(st[:, 0:M], pt[:])
                    ctxT_r.append(st)

                kT_r = []
                for p in range(NPAIR):
                    pt = ps.tile([128, 80], F, name=f"kps{p}", tag="misc")
                    for kt in range(6):
                        nc.tensor.matmul(pt[:], wk_r[kt][:, 128 * p:128 * (p + 1)],
                                         ctxT_r[kt][:], start=(kt == 0), stop=(kt == 5))
                    st = bp.tile([128, 80], R, name=f"kT{p}", tag=f"kT{p}")
                    nc.vector.tensor_copy(st[:], pt[:])
                    kT_r.append(st)

                v_ps = ps.tile([M, ID], F, name="v_ps", tag="misc")
                for kt in range(6):
                    nc.tensor.matmul(v_ps[:], ctxT_r[kt][:, 0:M], wv_r[kt][:],
                                     start=(kt == 0), stop=(kt == 5))
                v_r = bp.tile([M, H, 64], R, name="v_r", tag="v_r")
                nc.vector.memset(v_r[:, :, 40:64].bitcast(F), 0.0)
                nc.vector.tensor_copy(
                    v_r[:, :, 0:40],
                    v_ps[:].rearrange("p (h d) -> p h d", h=H)[:, :, 0:40])

                for c in range(NCHUNK):
                    t0 = CHUNK * c
                    # ---- x load + transpose ----
                    x_sb = cp.tile([128, 4, QD], R, name="x_sb", tag="x_sb")
                    nc.sync.dma_start(
                        x_sb[:],
                        x_d[b, t0:t0 + CHUNK, :].rearrange(
                            "(i p) cc -> p i cc", p=128).bitcast(R))
                    xT_r = []
                    for s, (o, w) in enumerate(CKT):
                        pt = ps.tile([w, CHUNK], F, name=f"xTps{s}", tag="misc")
                        for i in range(4):
                            nc.tensor.transpose(
                                pt[:, 128 * i:128 * (i + 1)].bitcast(R),
                                x_sb[:, i, o:o + w], id_r[:])
                        st = cp.tile([w, CHUNK], R, name=f"xT{s}", tag=f"xT{s}")
                        nc.vector.tensor_copy(st[:], pt[:])
                        xT_r.append(st)

                    # ---- q^T head pairs ----
                    qT_r = []
                    for p in range(NPAIR):
                        pt = ps.tile([128, CHUNK], F, name=f"qps{p}", tag="misc")
                        for kt in range(3):
                            nc.tensor.matmul(pt[:], wq_r[kt][:, 128 * p:128 * (p + 1)],
                                             xT_r[kt][:], start=(kt == 0), stop=(kt == 2))
                        st = cp.tile([128, CHUNK], R, name=f"qT{p}", tag=f"qT{p}")
                        nc.vector.tensor_copy(st[:], pt[:])
                        qT_r.append(st)

                    # ---- scores + exp per head ----
                    expS = []
                    for h in range(H):
                        p, j = h // 2, h % 2
                        base = 64 * j
                        spt = ps.tile([M, CHUNK], F, name=f"s{h}", tag="S")
                        nc.tensor.matmul(
                            spt[:], kT_r[p][base:base + DH, 0:M],
                            qT_r[p][base:base + DH, :], start=True, stop=True)
                        et = ep.tile([M, CHUNK], R, name=f"expS{h}", tag="expS")
                        nc.scalar.activation(et[:], spt[:], AF.Exp, scale=SCALE)
                        expS.append(et)

                    # ---- Zs = per-head sums via selector columns ----
                    zs_ps = ps.tile([H, CHUNK], F, name="zs_ps", tag="zsrb")
                    for h in range(H):
                        nc.tensor.matmul(zs_ps[:], z_r[:, H * h:H * (h + 1)],
                                         expS[h][:], start=(h == 0), stop=(h == H - 1))
                    lnz = cp.tile([H, CHUNK], F, name="lnz", tag="lnz")
                    nc.scalar.activation(lnz[:], zs_ps[:], AF.Ln)
                    rs_r = cp.tile([H, CHUNK], R, name="rs_r", tag="rs_r")
                    nc.scalar.activation(rs_r[:], lnz[:], AF.Exp, scale=-1.0)

                    # ---- O pairs (col-tiled) + normalize on copy ----
                    o_sb = []
                    for p in range(NPAIR):
                        opa = ps.tile([64, CHUNK], F, name=f"opa{p}", tag="O")
                        nc.tensor.matmul(opa[:], v_r[:, 2 * p, :],
                                         expS[2 * p][:], start=True, stop=True)
                        opb = ps.tile([64, CHUNK], F, name=f"opb{p}", tag="O")
                        nc.tensor.matmul(opb[:], v_r[:, 2 * p + 1, :],
                                         expS[2 * p + 1][:], start=True, stop=True)
                        rb_ps = ps.tile([128, CHUNK], F, name=f"rb{p}", tag="zsrb")
                        nc.tensor.matmul(rb_ps[:], e_r[:, 128 * p:128 * (p + 1)],
                                         rs_r[:], start=True, stop=True)
                        rb_sb = cp.tile([128, CHUNK], F, name=f"rbs{p}", tag="rb_sb")
                        nc.scalar.copy(rb_sb[:], rb_ps[:])
                        ot = cp.tile([128, CHUNK], R, name=f"osb{p}", tag=f"osb{p}")
                        nc.vector.tensor_tensor(out=ot[0:64, :], in0=opa[:],
                                                in1=rb_sb[0:64, :],
                                                op=mybir.AluOpType.mult)
                        nc.vector.tensor_tensor(out=ot[64:128, :], in0=opb[:],
                                                in1=rb_sb[64:128, :],
                                                op=mybir.AluOpType.mult)
                        o_sb.append(ot)

                    # ---- final projection + store ----
                    for i in range(4):
                        fp = ps.tile([128, QD], F, name=f"fin{i}", tag="misc")
                        for p in range(NPAIR):
                            nc.tensor.matmul(fp[:], o_sb[p][:, 128 * i:128 * (i + 1)],
                                             wo_r[p][:], start=(p == 0),
                                             stop=(p == NPAIR - 1 and not with_bias))
                        if with_bias:
                            nc.tensor.matmul(fp[:], ones_r[:], bo_r[:],
                                             start=False, stop=True)
                        ft = cp.tile([128, QD], F, name=f"fout{i}", tag=f"fout{i}")
                        nc.scalar.copy(ft[:], fp[:])
                        nc.sync.dma_start(
                            out_d[b, t0 + 128 * i: t0 + 128 * (i + 1), :], ft[:])

    _legalize_sync_waits(nc, mybir)
    return nc


def _get_module(with_bias):
    key = ("mod", with_bias)
    if key not in _cache:
        import concourse.bass as bass
        _cache[key] = _build(bass.Bass(), with_bias)
    return _cache[key]


def kernel(x, context, Wq, Wk, Wv, Wo, bo):
    from concourse.bass_utils import run_bass_kernel_spmd

    x = np.ascontiguousarray(x, dtype=np.float32)
    context = np.ascontiguousarray(context, dtype=np.float32)
    with_bias = bool(np.any(bo))
    aux = _aux_arrays(np.asarray(Wq, dtype=np.float32),
                      np.asarray(Wk, dtype=np.float32),
                      np.asarray(Wv, dtype=np.float32),
                      np.asarray(Wo, dtype=np.float32),
                      np.asarray(bo, dtype=np.float32))
    nc = _get_module(with_bias)

    in_maps = []
    for core in range(N_CORES):
        sl = slice(B_LOC * core, B_LOC * (core + 1))
        m = {"x": np.ascontiguousarray(x[sl]),
             "context": np.ascontiguousarray(context[sl]),
             "Wv": np.ascontiguousarray(np.asarray(Wv, dtype=np.float32))}
        m.update(aux)
        in_maps.append(m)

    res = run_bass_kernel_spmd(nc, in_maps, core_ids=list(range(N_CORES)))
    return np.concatenate([r["out"] for r in res.results], axis=0)


# revision 6
# speedup vs baseline: 1.4006x; 1.4006x over previous
"""Cross-attention kernel for 8 Trainium2 NeuronCores.

Problem: out = softmax((x@Wq)(ctx@Wk)^T * dh^-0.5) @ (ctx@Wv) @ Wo + bo
  x [16, 4096, 320], ctx [16, 77, 768], H=8 heads x DH=40.

Sharding: data-parallel over batch (2 per core), SPMD one NEFF.

Per-core layout ("^T domain": features on partitions, tokens on free dim):
  x^T tiles  <- PE transpose of x
  q^T        <- Wq_pad.T @ x^T      (head pairs 64-aligned: rows 0:40 / 64:104)
  S_h        <- k^T_h.T @ q^T_h     [77 ctx-tok, 512 q-tok] per head
  P_h        <- exp(S_h * scale)    (ACT, fp32r out)
  Zs         <- accumulated selector-column matmuls  [8, 512] (row h = sum_j P_h)
  Rs         <- exp(-ln(Zs))        (ACT; 1/Z without the banned reciprocal)
  Rb         <- E_pair.T @ Rs       broadcast R rows to head-pair layout
  O_pair     <- v_h.T @ P_h (col-tiled pairs), copy*Rb -> SBUF (normalized)
  out        <- O_pair.T @ Wo_pad (+ bo) -> [tokens, 320] -> DMA out

All matmuls run float32r (~1.5e-4 rel err, 1 cycle/row at N>=256).
"""

import numpy as np

H, DH = 8, 40
SCALE = DH ** -0.5
B, N, M = 16, 4096, 77
QD, CD, ID = 320, 768, H * DH
N_CORES = 8
B_LOC = B // N_CORES
CHUNK = 512
NCHUNK = N // CHUNK
NPAIR = H // 2  # head pairs, 64-aligned rows {0:40, 64:104}

_cache = {}


def _legalize_sync_waits(nc, mybir):
    """This walrus build allows 1 sync-wait command per instruction (2 for
    EventSemaphore). Spill extra waits onto same-engine NoOps placed just
    before; per-engine program order makes that equivalent."""
    n = 0
    f = nc.m.functions[0]
    for blk in f.blocks:
        out = []
        changed = False
        for inst in blk.instructions:
            si = inst.sync_info
            waits = list(si.on_wait) if si is not None and si.on_wait else []
            cap = 2 if isinstance(inst, mybir.InstEventSemaphore) else 1
            if len(waits) > cap:
                keep, spill = waits[-cap:], waits[:-cap]
                for w in spill:
                    n += 1
                    nop = mybir.InstNoOp(name=f"I-waitfix-{n}", ins=[], outs=[],
                                         engine=inst.engine)
                    nop.sync_info = mybir.SyncInfo(on_wait=[w], on_update=[])
                    out.append(nop)
                inst.sync_info = mybir.SyncInfo(
                    on_wait=keep,
                    on_update=list(si.on_update) if si.on_update else [])
                changed = True
            out.append(inst)
        if changed:
            blk.instructions = out
    return n


def _aux_arrays(Wq, Wk, Wv, Wo, bo):
    """Host-side constant prep: pad head pairs to 64-aligned rows."""
    def pad_pairs_cols(W):
        # W [c, 320] -> [c, 4, 128]: cols 0:40 = head 2p, 64:104 = head 2p+1
        c = W.shape[0]
        out = np.zeros((c, NPAIR, 128), dtype=np.float32)
        for p in range(NPAIR):
            out[:, p, 0:40] = W[:, 80 * p: 80 * p + 40]
            out[:, p, 64:104] = W[:, 80 * p + 40: 80 * p + 80]
        return out.reshape(c, NPAIR * 128)

    wq_pad = pad_pairs_cols(Wq)                      # [320, 512]
    wk_pad = pad_pairs_cols(Wk)                      # [768, 512]
    # Wo rows padded to pair layout: [4, 128, 320]
    wo_pad = np.zeros((NPAIR, 128, QD), dtype=np.float32)
    for p in range(NPAIR):
        wo_pad[p, 0:40] = Wo[80 * p: 80 * p + 40]
        wo_pad[p, 64:104] = Wo[80 * p + 40: 80 * p + 80]
    # E: [8, 4*128] broadcast map R_h -> pair rows
    e_mat = np.zeros((H, NPAIR * 128), dtype=np.float32)
    for p in range(NPAIR):
        e_mat[2 * p, 128 * p: 128 * p + 40] = 1.0
        e_mat[2 * p + 1, 128 * p + 64: 128 * p + 104] = 1.0
    # selector columns for Zs accumulation: [77, 8], col h = ones
    zcol = np.zeros((M, H, H), dtype=np.float32)
    for h in range(H):
        zcol[:, h, h] = 1.0
    zcol = zcol.reshape(M, H * H)
    ident = np.eye(128, dtype=np.float32)
    return {
        "aux_wq": wq_pad, "aux_wk": wk_pad, "aux_wo": wo_pad.reshape(NPAIR * 128, QD),
        "aux_e": e_mat, "aux_z": zcol, "aux_id": ident,
        "aux_bo": bo.reshape(1, QD).astype(np.float32),
    }


def _build(nc, with_bias):
    import concourse.mybir as mybir
    from concourse.tile import TileContext

    R = mybir.dt.float32r
    F = mybir.dt.float32
    AF = mybir.ActivationFunctionType

    x_d = nc.dram_tensor("x", [B_LOC, N, QD], F, kind="ExternalInput")
    c_d = nc.dram_tensor("context", [B_LOC, M, CD], F, kind="ExternalInput")
    wq_d = nc.dram_tensor("aux_wq", [QD, 512], F, kind="ExternalInput")
    wk_d = nc.dram_tensor("aux_wk", [CD, 512], F, kind="ExternalInput")
    wv_d = nc.dram_tensor("Wv", [CD, ID], F, kind="ExternalInput")
    wo_d = nc.dram_tensor("aux_wo", [512, QD], F, kind="ExternalInput")
    e_d = nc.dram_tensor("aux_e", [H, 512], F, kind="ExternalInput")
    z_d = nc.dram_tensor("aux_z", [M, H * H], F, kind="ExternalInput")
    id_d = nc.dram_tensor("aux_id", [128, 128], F, kind="ExternalInput")
    bo_d = nc.dram_tensor("aux_bo", [1, QD], F, kind="ExternalInput")
    out_d = nc.dram_tensor("out", [B_LOC, N, QD], F, kind="ExternalOutput")

    CKT = [(0, 128), (128, 128), (256, 64)]            # QD k-tiles
    CKT6 = [(128 * i, 128) for i in range(6)]          # CD k-tiles

    with TileContext(nc) as tc:
        with tc.tile_pool(name="wpool", bufs=1) as wp, \
             tc.tile_pool(name="bpool", bufs=2) as bp, \
             tc.tile_pool(name="cpool", bufs=2) as cp, \
             tc.tile_pool(name="epool", bufs=10) as ep, \
             tc.tile_pool(name="ps", bufs=2, space="PSUM") as ps:

            # ---- per-core constants ----
            wq_r = []
            for kt, (o, w) in enumerate(CKT):
                t = wp.tile([w, 512], R, name=f"wq{kt}", tag=f"wq{kt}")
                nc.sync.dma_start(t[:], wq_d[o:o + w, :].bitcast(R))
                wq_r.append(t)
            wk_r = []
            wv_r = []
            for kt, (o, w) in enumerate(CKT6):
                t = wp.tile([w, 512], R, name=f"wk{kt}", tag=f"wk{kt}")
                nc.sync.dma_start(t[:], wk_d[o:o + w, :].bitcast(R))
                wk_r.append(t)
                t2 = wp.tile([w, ID], R, name=f"wv{kt}", tag=f"wv{kt}")
                nc.sync.dma_start(t2[:], wv_d[o:o + w, :].bitcast(R))
                wv_r.append(t2)
            wo_r = []
            for p in range(NPAIR):
                t = wp.tile([128, QD], R, name=f"wo{p}", tag=f"wo{p}")
                nc.sync.dma_start(t[:], wo_d[128 * p:128 * p + 128, :].bitcast(R))
                wo_r.append(t)
            e_r = wp.tile([H, 512], R, name="e_r", tag="e_r")
            nc.sync.dma_start(e_r[:], e_d[:].bitcast(R))
            z_r = wp.tile([M, H * H], R, name="z_r", tag="z_r")
            nc.sync.dma_start(z_r[:], z_d[:].bitcast(R))
            id_r = wp.tile([128, 128], R, name="id_r", tag="id_r")
            nc.sync.dma_start(id_r[:], id_d[:].bitcast(R))
            id_f = wp.tile([128, 128], F, name="id_f", tag="id_f")
            nc.sync.dma_start(id_f[:], id_d[:])
            if with_bias:
                bo_r = wp.tile([1, QD], R, name="bo_r", tag="bo_r")
                nc.sync.dma_start(bo_r[:], bo_d[:].bitcast(R))
                ones_r = wp.tile([1, 128], R, name="ones_r", tag="ones_r")
                nc.vector.memset(ones_r[:].bitcast(F), 1.0)

            for b in range(B_LOC):
                # ---- per-batch: ctx^T, k^T pairs, v ----
                ctx_f = bp.tile([M, CD], F, name="ctx_f", tag="ctx")
                nc.sync.dma_start(ctx_f[:], c_d[b])
                ctxT_r = []
                for s, (o, w) in enumerate(CKT6):
                    pt = ps.tile([w, M], F, name=f"ctT{s}", tag="xtq")
                    nc.tensor.transpose(pt[:], ctx_f[:, o:o + w], id_f[:M, :M])
                    st = bp.tile([w, 80], R, name=f"ctxT{s}", tag=f"ctxT{s}")
                    nc.vector.memset(st[:, M:80].bitcast(F), 0.0)
                    nc.vector.tensor_copy(st[:, 0:M], pt[:])
                    ctxT_r.append(st)

                kT_r = []
                for p in range(NPAIR):
                    pt = ps.tile([128, 80], F, name=f"kps{p}", tag="xtq")
                    for kt in range(6):
                        nc.tensor.matmul(pt[:], wk_r[kt][:, 128 * p:128 * (p + 1)],
                                         ctxT_r[kt][:], start=(kt == 0), stop=(kt == 5))
                    st = bp.tile([128, 80], R, name=f"kT{p}", tag=f"kT{p}")
                    nc.vector.tensor_copy(st[:], pt[:])
                    kT_r.append(st)

                v_ps = ps.tile([M, ID], F, name="v_ps", tag="xtq")
                for kt in range(6):
                    nc.tensor.matmul(v_ps[:], ctxT_r[kt][:, 0:M], wv_r[kt][:],
                                     start=(kt == 0), stop=(kt == 5))
                v_r = bp.tile([M, H, 64], R, name="v_r", tag="v_r")
                nc.vector.memset(v_r[:, :, 40:64].bitcast(F), 0.0)
                nc.vector.tensor_copy(
                    v_r[:, :, 0:40],
                    v_ps[:].rearrange("p (h d) -> p h d", h=H)[:, :, 0:40])

                for c in range(NCHUNK):
                    t0 = CHUNK * c
                    # ---- x load + transpose ----
                    x_sb = cp.tile([128, 4, QD], R, name="x_sb", tag="x_sb")
                    nc.sync.dma_start(
                        x_sb[:],
                        x_d[b, t0:t0 + CHUNK, :].rearrange(
                            "(i p) cc -> p i cc", p=128).bitcast(R))
                    xT_r = []
                    for s, (o, w) in enumerate(CKT):
                        pt = ps.tile([w, CHUNK], F, name=f"xTps{s}", tag="xtq")
                        for i in range(4):
                            nc.tensor.transpose(
                                pt[:, 128 * i:128 * (i + 1)].bitcast(R),
                                x_sb[:, i, o:o + w], id_r[:])
                        st = cp.tile([w, CHUNK], R, name=f"xT{s}", tag=f"xT{s}")
                        nc.vector.tensor_copy(st[:], pt[:])
                        xT_r.append(st)

                    # ---- q^T head pairs ----
                    qT_r = []
                    for p in range(NPAIR):
                        pt = ps.tile([128, CHUNK], F, name=f"qps{p}", tag="xtq")
                        for kt in range(3):
                            nc.tensor.matmul(pt[:], wq_r[kt][:, 128 * p:128 * (p + 1)],
                                             xT_r[kt][:], start=(kt == 0), stop=(kt == 2))
                        st = cp.tile([128, CHUNK], R, name=f"qT{p}", tag=f"qT{p}")
                        nc.vector.tensor_copy(st[:], pt[:])
                        qT_r.append(st)

                    # ---- scores + exp per head ----
                    expS = []
                    for h in range(H):
                        p, j = h // 2, h % 2
                        base = 64 * j
                        spt = ps.tile([M, CHUNK], F, name=f"s{h}", tag="S")
                        nc.tensor.matmul(
                            spt[:], kT_r[p][base:base + DH, 0:M],
                            qT_r[p][base:base + DH, :], start=True, stop=True)
                        et = ep.tile([M, CHUNK], R, name=f"expS{h}", tag="expS")
                        nc.scalar.activation(et[:], spt[:], AF.Exp, scale=SCALE)
                        expS.append(et)

                    # ---- Zs = per-head sums via selector columns ----
                    zs_ps = ps.tile([H, CHUNK], F, name="zs_ps", tag="fz")
                    for h in range(H):
                        nc.tensor.matmul(zs_ps[:], z_r[:, H * h:H * (h + 1)],
                                         expS[h][:], start=(h == 0), stop=(h == H - 1))
                    lnz = cp.tile([H, CHUNK], F, name="lnz", tag="lnz")
                    nc.scalar.activation(lnz[:], zs_ps[:], AF.Ln)
                    rs_r = cp.tile([H, CHUNK], R, name="rs_r", tag="rs_r")
                    nc.scalar.activation(rs_r[:], lnz[:], AF.Exp, scale=-1.0)

                    # ---- O pairs (col-tiled) + normalize on copy ----
                    o_sb = []
                    for p in range(NPAIR):
                        opa = ps.tile([64, CHUNK], F, name=f"opa{p}", tag="O")
                        nc.tensor.matmul(opa[:], v_r[:, 2 * p, :],
                                         expS[2 * p][:], start=True, stop=True)
                        opb = ps.tile([64, CHUNK], F, name=f"opb{p}", tag="O")
                        nc.tensor.matmul(opb[:], v_r[:, 2 * p + 1, :],
                                         expS[2 * p + 1][:], start=True, stop=True)
                        rb_ps = ps.tile([128, CHUNK], F, name=f"rb{p}", tag="fz")
                        nc.tensor.matmul(rb_ps[:], e_r[:, 128 * p:128 * (p + 1)],
                                         rs_r[:], start=True, stop=True)
                        rb_sb = cp.tile([128, CHUNK], F, name=f"rbs{p}", tag="rb_sb")
                        nc.scalar.copy(rb_sb[:], rb_ps[:])
                        ot = cp.tile([128, CHUNK], R, name=f"osb{p}", tag=f"osb{p}")
                        nc.vector.tensor_tensor(out=ot[0:64, :], in0=opa[:],
                                                in1=rb_sb[0:64, :],
                                                op=mybir.AluOpType.mult)
                        nc.vector.tensor_tensor(out=ot[64:128, :], in0=opb[:],
                                                in1=rb_sb[64:128, :],
                                                op=mybir.AluOpType.mult)
                        o_sb.append(ot)

                    # ---- final projection + store ----
                    for i in range(4):
                        fp = ps.tile([128, QD], F, name=f"fin{i}", tag="fz")
                        for p in range(NPAIR):
                            nc.tensor.matmul(fp[:], o_sb[p][:, 128 * i:128 * (i + 1)],
                                             wo_r[p][:], start=(p == 0),
                                             stop=(p == NPAIR - 1 and not with_bias))
                        if with_bias:
                            nc.tensor.matmul(fp[:], ones_r[:], bo_r[:],
                                             start=False, stop=True)
                        ft = cp.tile([128, QD], F, name=f"fout{i}", tag=f"fout{i}")
                        nc.scalar.copy(ft[:], fp[:])
                        nc.sync.dma_start(
                            out_d[b, t0 + 128 * i: t0 + 128 * (i + 1), :], ft[:])

    _legalize_sync_waits(nc, mybir)
    return nc


def _get_module(with_bias):
    key = ("mod", with_bias)
    if key not in _cache:
        import concourse.bass as bass
        _cache[key] = _build(bass.Bass(), with_bias)
    return _cache[key]


def kernel(x, context, Wq, Wk, Wv, Wo, bo):
    from concourse.bass_utils import run_bass_kernel_spmd

    x = np.ascontiguousarray(x, dtype=np.float32)
    context = np.ascontiguousarray(context, dtype=np.float32)
    with_bias = bool(np.any(bo))
    aux = _aux_arrays(np.asarray(Wq, dtype=np.float32),
                      np.asarray(Wk, dtype=np.float32),
                      np.asarray(Wv, dtype=np.float32),
                      np.asarray(Wo, dtype=np.float32),
                      np.asarray(bo, dtype=np.float32))
    nc = _get_module(with_bias)

    in_maps = []
    for core in range(N_CORES):
        sl = slice(B_LOC * core, B_LOC * (core + 1))
        m = {"x": np.ascontiguousarray(x[sl]),
             "context": np.ascontiguousarray(context[sl]),
             "Wv": np.ascontiguousarray(np.asarray(Wv, dtype=np.float32))}
        m.update(aux)
        in_maps.append(m)

    res = run_bass_kernel_spmd(nc, in_maps, core_ids=list(range(N_CORES)))
    return np.concatenate([r["out"] for r in res.results], axis=0)


# revision 7
# speedup vs baseline: 1.4219x; 1.0152x over previous
"""Cross-attention kernel for 8 Trainium2 NeuronCores.

Problem: out = softmax((x@Wq)(ctx@Wk)^T * dh^-0.5) @ (ctx@Wv) @ Wo + bo
  x [16, 4096, 320], ctx [16, 77, 768], H=8 heads x DH=40.

Sharding: data-parallel over batch (2 per core), SPMD one NEFF.

Per-core layout ("^T domain": features on partitions, tokens on free dim):
  x^T tiles  <- PE transpose of x
  q^T        <- Wq_pad.T @ x^T      (head pairs 64-aligned: rows 0:40 / 64:104)
  S_h        <- k^T_h.T @ q^T_h     [77 ctx-tok, 512 q-tok] per head
  P_h        <- exp(S_h * scale)    (ACT, fp32r out)
  Zs         <- accumulated selector-column matmuls  [8, 512] (row h = sum_j P_h)
  Rs         <- exp(-ln(Zs))        (ACT; 1/Z without the banned reciprocal)
  Rb         <- E_pair.T @ Rs       broadcast R rows to head-pair layout
  O_pair     <- v_h.T @ P_h (col-tiled pairs), copy*Rb -> SBUF (normalized)
  out        <- O_pair.T @ Wo_pad (+ bo) -> [tokens, 320] -> DMA out

All matmuls run float32r (~1.5e-4 rel err, 1 cycle/row at N>=256).
"""

import numpy as np

H, DH = 8, 40
SCALE = DH ** -0.5
B, N, M = 16, 4096, 77
QD, CD, ID = 320, 768, H * DH
N_CORES = 8
B_LOC = B // N_CORES
CHUNK = 512
NCHUNK = N // CHUNK
NPAIR = H // 2  # head pairs, 64-aligned rows {0:40, 64:104}

_cache = {}


def _legalize_sync_waits(nc, mybir):
    """This walrus build allows 1 sync-wait command per instruction (2 for
    EventSemaphore). Spill extra waits onto same-engine NoOps placed just
    before; per-engine program order makes that equivalent."""
    n = 0
    f = nc.m.functions[0]
    for blk in f.blocks:
        out = []
        changed = False
        for inst in blk.instructions:
            si = inst.sync_info
            waits = list(si.on_wait) if si is not None and si.on_wait else []
            cap = 2 if isinstance(inst, mybir.InstEventSemaphore) else 1
            if len(waits) > cap:
                keep, spill = waits[-cap:], waits[:-cap]
                for w in spill:
                    n += 1
                    nop = mybir.InstNoOp(name=f"I-waitfix-{n}", ins=[], outs=[],
                                         engine=inst.engine)
                    nop.sync_info = mybir.SyncInfo(on_wait=[w], on_update=[])
                    out.append(nop)
                inst.sync_info = mybir.SyncInfo(
                    on_wait=keep,
                    on_update=list(si.on_update) if si.on_update else [])
                changed = True
            out.append(inst)
        if changed:
            blk.instructions = out
    return n


def _aux_arrays(Wq, Wk, Wv, Wo, bo):
    """Host-side constant prep: pad head pairs to 64-aligned rows."""
    def pad_pairs_cols(W):
        # W [c, 320] -> [c, 4, 128]: cols 0:40 = head 2p, 64:104 = head 2p+1
        c = W.shape[0]
        out = np.zeros((c, NPAIR, 128), dtype=np.float32)
        for p in range(NPAIR):
            out[:, p, 0:40] = W[:, 80 * p: 80 * p + 40]
            out[:, p, 64:104] = W[:, 80 * p + 40: 80 * p + 80]
        return out.reshape(c, NPAIR * 128)

    wq_pad = pad_pairs_cols(Wq)                      # [320, 512]
    wk_pad = pad_pairs_cols(Wk)                      # [768, 512]
    # Wo rows padded to pair layout: [4, 128, 320]
    wo_pad = np.zeros((NPAIR, 128, QD), dtype=np.float32)
    for p in range(NPAIR):
        wo_pad[p, 0:40] = Wo[80 * p: 80 * p + 40]
        wo_pad[p, 64:104] = Wo[80 * p + 40: 80 * p + 80]
    # E: [8, 4*128] broadcast map R_h -> pair rows
    e_mat = np.zeros((H, NPAIR * 128), dtype=np.float32)
    for p in range(NPAIR):
        e_mat[2 * p, 128 * p: 128 * p + 40] = 1.0
        e_mat[2 * p + 1, 128 * p + 64: 128 * p + 104] = 1.0
    # selector columns for Zs accumulation: [77, 8], col h = ones
    zcol = np.zeros((M, H, H), dtype=np.float32)
    for h in range(H):
        zcol[:, h, h] = 1.0
    zcol = zcol.reshape(M, H * H)
    ident = np.eye(128, dtype=np.float32)
    return {
        "aux_wq": wq_pad, "aux_wk": wk_pad, "aux_wo": wo_pad.reshape(NPAIR * 128, QD),
        "aux_e": e_mat, "aux_z": zcol, "aux_id": ident,
        "aux_bo": bo.reshape(1, QD).astype(np.float32),
    }


def _build(nc, with_bias):
    import concourse.mybir as mybir
    from concourse.tile import TileContext

    R = mybir.dt.float32r
    F = mybir.dt.float32
    AF = mybir.ActivationFunctionType

    x_d = nc.dram_tensor("x", [B_LOC, N, QD], F, kind="ExternalInput")
    c_d = nc.dram_tensor("context", [B_LOC, M, CD], F, kind="ExternalInput")
    wq_d = nc.dram_tensor("aux_wq", [QD, 512], F, kind="ExternalInput")
    wk_d = nc.dram_tensor("aux_wk", [CD, 512], F, kind="ExternalInput")
    wv_d = nc.dram_tensor("Wv", [CD, ID], F, kind="ExternalInput")
    wo_d = nc.dram_tensor("aux_wo", [512, QD], F, kind="ExternalInput")
    e_d = nc.dram_tensor("aux_e", [H, 512], F, kind="ExternalInput")
    z_d = nc.dram_tensor("aux_z", [M, H * H], F, kind="ExternalInput")
    id_d = nc.dram_tensor("aux_id", [128, 128], F, kind="ExternalInput")
    bo_d = nc.dram_tensor("aux_bo", [1, QD], F, kind="ExternalInput")
    out_d = nc.dram_tensor("out", [B_LOC, N, QD], F, kind="ExternalOutput")

    CKT = [(0, 128), (128, 128), (256, 64)]            # QD k-tiles
    CKT6 = [(128 * i, 128) for i in range(6)]          # CD k-tiles

    with TileContext(nc) as tc:
        with tc.tile_pool(name="wpool", bufs=1) as wp, \
             tc.tile_pool(name="bpool", bufs=2) as bp, \
             tc.tile_pool(name="cpool", bufs=2) as cp, \
             tc.tile_pool(name="epool", bufs=10) as ep, \
             tc.tile_pool(name="ps", bufs=2, space="PSUM") as ps:

            # ---- per-core constants ----
            wq_r = []
            for kt, (o, w) in enumerate(CKT):
                t = wp.tile([w, 512], R, name=f"wq{kt}", tag=f"wq{kt}")
                nc.sync.dma_start(t[:], wq_d[o:o + w, :].bitcast(R))
                wq_r.append(t)
            wk_r = []
            wv_r = []
            for kt, (o, w) in enumerate(CKT6):
                t = wp.tile([w, 512], R, name=f"wk{kt}", tag=f"wk{kt}")
                nc.sync.dma_start(t[:], wk_d[o:o + w, :].bitcast(R))
                wk_r.append(t)
                t2 = wp.tile([w, ID], R, name=f"wv{kt}", tag=f"wv{kt}")
                nc.sync.dma_start(t2[:], wv_d[o:o + w, :].bitcast(R))
                wv_r.append(t2)
            wo_r = []
            for p in range(NPAIR):
                t = wp.tile([128, QD], R, name=f"wo{p}", tag=f"wo{p}")
                nc.sync.dma_start(t[:], wo_d[128 * p:128 * p + 128, :].bitcast(R))
                wo_r.append(t)
            e_r = wp.tile([H, 512], R, name="e_r", tag="e_r")
            nc.sync.dma_start(e_r[:], e_d[:].bitcast(R))
            z_r = wp.tile([M, H * H], R, name="z_r", tag="z_r")
            nc.sync.dma_start(z_r[:], z_d[:].bitcast(R))
            id_r = wp.tile([128, 128], R, name="id_r", tag="id_r")
            nc.sync.dma_start(id_r[:], id_d[:].bitcast(R))
            id_f = wp.tile([128, 128], F, name="id_f", tag="id_f")
            nc.sync.dma_start(id_f[:], id_d[:])
            if with_bias:
                bo_r = wp.tile([1, QD], R, name="bo_r", tag="bo_r")
                nc.sync.dma_start(bo_r[:], bo_d[:].bitcast(R))
                ones_r = wp.tile([1, 128], R, name="ones_r", tag="ones_r")
                nc.vector.memset(ones_r[:].bitcast(F), 1.0)

            for b in range(B_LOC):
                # ---- per-batch: ctx^T, k^T pairs, v ----
                ctx_f = bp.tile([M, CD], F, name="ctx_f", tag="ctx")
                nc.sync.dma_start(ctx_f[:], c_d[b])
                ctxT_r = []
                for s, (o, w) in enumerate(CKT6):
                    pt = ps.tile([w, M], F, name=f"ctT{s}", tag="xtq")
                    nc.tensor.transpose(pt[:], ctx_f[:, o:o + w], id_f[:M, :M])
                    st = bp.tile([w, 80], R, name=f"ctxT{s}", tag=f"ctxT{s}")
                    nc.vector.memset(st[:, M:80].bitcast(F), 0.0)
                    nc.vector.tensor_copy(st[:, 0:M], pt[:])
                    ctxT_r.append(st)

                kT_r = []
                for p in range(NPAIR):
                    pt = ps.tile([128, 80], F, name=f"kps{p}", tag="xtq")
                    for kt in range(6):
                        nc.tensor.matmul(pt[:], wk_r[kt][:, 128 * p:128 * (p + 1)],
                                         ctxT_r[kt][:], start=(kt == 0), stop=(kt == 5))
                    st = bp.tile([128, 80], R, name=f"kT{p}", tag=f"kT{p}")
                    nc.vector.tensor_copy(st[:], pt[:])
                    kT_r.append(st)

                v_ps = ps.tile([M, ID], F, name="v_ps", tag="xtq")
                for kt in range(6):
                    nc.tensor.matmul(v_ps[:], ctxT_r[kt][:, 0:M], wv_r[kt][:],
                                     start=(kt == 0), stop=(kt == 5))
                v_r = bp.tile([M, H, 64], R, name="v_r", tag="v_r")
                nc.vector.memset(v_r[:, :, 40:64].bitcast(F), 0.0)
                nc.vector.tensor_copy(
                    v_r[:, :, 0:40],
                    v_ps[:].rearrange("p (h d) -> p h d", h=H)[:, :, 0:40])

                for c in range(NCHUNK):
                    t0 = CHUNK * c
                    # ---- x load + transpose ----
                    x_sb = cp.tile([128, 4, QD], R, name="x_sb", tag="x_sb")
                    nc.sync.dma_start(
                        x_sb[:],
                        x_d[b, t0:t0 + CHUNK, :].rearrange(
                            "(i p) cc -> p i cc", p=128).bitcast(R))
                    xT_r = []
                    for s, (o, w) in enumerate(CKT):
                        pt = ps.tile([w, CHUNK], F, name=f"xTps{s}", tag="xtq")
                        for i in range(4):
                            nc.tensor.transpose(
                                pt[:, 128 * i:128 * (i + 1)].bitcast(R),
                                x_sb[:, i, o:o + w], id_r[:])
                        st = cp.tile([w, CHUNK], R, name=f"xT{s}", tag=f"xT{s}")
                        nc.vector.tensor_copy(st[:], pt[:])
                        xT_r.append(st)

                    # ---- q^T head pairs ----
                    qT_r = []
                    for p in range(NPAIR):
                        pt = ps.tile([128, CHUNK], F, name=f"qps{p}", tag="xtq")
                        for kt in range(3):
                            nc.tensor.matmul(pt[:], wq_r[kt][:, 128 * p:128 * (p + 1)],
                                             xT_r[kt][:], start=(kt == 0), stop=(kt == 2))
                        st = cp.tile([128, CHUNK], R, name=f"qT{p}", tag=f"qT{p}")
                        nc.vector.tensor_copy(st[:], pt[:])
                        qT_r.append(st)

                    # ---- scores + exp per head ----
                    expS = []
                    for h in range(H):
                        p, j = h // 2, h % 2
                        base = 64 * j
                        spt = ps.tile([M, CHUNK], F, name=f"s{h}", tag="S")
                        nc.tensor.matmul(
                            spt[:], kT_r[p][base:base + DH, 0:M],
                            qT_r[p][base:base + DH, :], start=True, stop=True)
                        et = ep.tile([M, CHUNK], R, name=f"expS{h}", tag="expS")
                        nc.scalar.activation(et[:], spt[:], AF.Exp, scale=SCALE)
                        expS.append(et)

                    # ---- Zs = per-head sums via selector columns ----
                    zs_ps = ps.tile([H, CHUNK], F, name="zs_ps", tag="fz")
                    for h in range(H):
                        nc.tensor.matmul(zs_ps[:], z_r[:, H * h:H * (h + 1)],
                                         expS[h][:], start=(h == 0), stop=(h == H - 1))
                    lnz = cp.tile([H, CHUNK], F, name="lnz", tag="lnz")
                    nc.scalar.activation(lnz[:], zs_ps[:], AF.Ln)
                    rs_r = cp.tile([H, CHUNK], R, name="rs_r", tag="rs_r")
                    nc.scalar.activation(rs_r[:], lnz[:], AF.Exp, scale=-1.0)

                    # ---- O pairs (col-tiled) + normalize on copy ----
                    o_sb = []
                    for p in range(NPAIR):
                        opa = ps.tile([64, CHUNK], F, name=f"opa{p}", tag="O")
                        nc.tensor.matmul(opa[:], v_r[:, 2 * p, :],
                                         expS[2 * p][:], start=True, stop=True)
                        opb = ps.tile([64, CHUNK], F, name=f"opb{p}", tag="O")
                        nc.tensor.matmul(opb[:], v_r[:, 2 * p + 1, :],
                                         expS[2 * p + 1][:], start=True, stop=True)
                        rb_ps = ps.tile([128, CHUNK], F, name=f"rb{p}", tag="fz")
                        nc.tensor.matmul(rb_ps[:], e_r[:, 128 * p:128 * (p + 1)],
                                         rs_r[:], start=True, stop=True)
                        rb_sb = cp.tile([128, CHUNK], F, name=f"rbs{p}", tag="rb_sb")
                        nc.scalar.copy(rb_sb[:], rb_ps[:])
                        ot = cp.tile([128, CHUNK], R, name=f"osb{p}", tag=f"osb{p}")
                        nc.vector.tensor_tensor(out=ot[0:64, :], in0=opa[:],
                                                in1=rb_sb[0:64, :],
                                                op=mybir.AluOpType.mult)
                        nc.vector.tensor_tensor(out=ot[64:128, :], in0=opb[:],
                                                in1=rb_sb[64:128, :],
                                                op=mybir.AluOpType.mult)
                        o_sb.append(ot)

                    # ---- final projection + store ----
                    for i in range(4):
                        fp = ps.tile([128, QD], F, name=f"fin{i}", tag="fz")
                        for p in range(NPAIR):
                            nc.tensor.matmul(fp[:], o_sb[p][:, 128 * i:128 * (i + 1)],
                                             wo_r[p][:], start=(p == 0),
                                             stop=(p == NPAIR - 1 and not with_bias))
                        if with_bias:
                            nc.tensor.matmul(fp[:], ones_r[:], bo_r[:],
                                             start=False, stop=True)
                        ft = cp.tile([128, QD], F, name=f"fout{i}", tag=f"fout{i}")
                        nc.vector.tensor_copy(ft[:], fp[:])
                        nc.sync.dma_start(
                            out_d[b, t0 + 128 * i: t0 + 128 * (i + 1), :], ft[:])

    _legalize_sync_waits(nc, mybir)
    return nc


def _get_module(with_bias):
    key = ("mod", with_bias)
    if key not in _cache:
        import concourse.bass as bass
        _cache[key] = _build(bass.Bass(), with_bias)
    return _cache[key]


def kernel(x, context, Wq, Wk, Wv, Wo, bo):
    import os
    import sys
    if os.environ.get("JAX_PLATFORMS") == "cpu" and "jax" not in sys.modules:
        del os.environ["JAX_PLATFORMS"]
    from concourse.bass_utils import run_bass_kernel_spmd

    x = np.ascontiguousarray(x, dtype=np.float32)
    context = np.ascontiguousarray(context, dtype=np.float32)
    with_bias = bool(np.any(bo))
    aux = _aux_arrays(np.asarray(Wq, dtype=np.float32),
                      np.asarray(Wk, dtype=np.float32),
                      np.asarray(Wv, dtype=np.float32),
                      np.asarray(Wo, dtype=np.float32),
                      np.asarray(bo, dtype=np.float32))
    nc = _get_module(with_bias)

    in_maps = []
    for core in range(N_CORES):
        sl = slice(B_LOC * core, B_LOC * (core + 1))
        m = {"x": np.ascontiguousarray(x[sl]),
             "context": np.ascontiguousarray(context[sl]),
             "Wv": np.ascontiguousarray(np.asarray(Wv, dtype=np.float32))}
        m.update(aux)
        in_maps.append(m)

    res = run_bass_kernel_spmd(nc, in_maps, core_ids=list(range(N_CORES)))
    return np.concatenate([r["out"] for r in res.results], axis=0)


# revision 8
# speedup vs baseline: 1.4416x; 1.0139x over previous
"""Cross-attention kernel for 8 Trainium2 NeuronCores.

Problem: out = softmax((x@Wq)(ctx@Wk)^T * dh^-0.5) @ (ctx@Wv) @ Wo + bo
  x [16, 4096, 320], ctx [16, 77, 768], H=8 heads x DH=40.

Sharding: data-parallel over batch (2 per core), SPMD one NEFF.

Per-core layout ("^T domain": features on partitions, tokens on free dim):
  x^T tiles  <- PE transpose of x
  q^T        <- Wq_pad.T @ x^T      (head pairs 64-aligned: rows 0:40 / 64:104)
  S_h        <- k^T_h.T @ q^T_h     [77 ctx-tok, 512 q-tok] per head
  P_h        <- exp(S_h * scale)    (ACT, fp32r out)
  Zs         <- accumulated selector-column matmuls  [8, 512] (row h = sum_j P_h)
  Rs         <- exp(-ln(Zs))        (ACT; 1/Z without the banned reciprocal)
  Rb         <- E_pair.T @ Rs       broadcast R rows to head-pair layout
  O_pair     <- v_h.T @ P_h (col-tiled pairs), copy*Rb -> SBUF (normalized)
  out        <- O_pair.T @ Wo_pad (+ bo) -> [tokens, 320] -> DMA out

All matmuls run float32r (~1.5e-4 rel err, 1 cycle/row at N>=256).
"""

import numpy as np

H, DH = 8, 40
SCALE = DH ** -0.5
B, N, M = 16, 4096, 77
QD, CD, ID = 320, 768, H * DH
N_CORES = 8
B_LOC = B // N_CORES
CHUNK = 512
NCHUNK = N // CHUNK
NPAIR = H // 2  # head pairs, 64-aligned rows {0:40, 64:104}

_cache = {}


def _legalize_sync_waits(nc, mybir):
    """This walrus build allows 1 sync-wait command per instruction (2 for
    EventSemaphore). Spill extra waits onto same-engine NoOps placed just
    before; per-engine program order makes that equivalent."""
    n = 0
    f = nc.m.functions[0]
    for blk in f.blocks:
        out = []
        changed = False
        for inst in blk.instructions:
            si = inst.sync_info
            waits = list(si.on_wait) if si is not None and si.on_wait else []
            cap = 2 if isinstance(inst, mybir.InstEventSemaphore) else 1
            if len(waits) > cap:
                keep, spill = waits[-cap:], waits[:-cap]
                for w in spill:
                    n += 1
                    nop = mybir.InstNoOp(name=f"I-waitfix-{n}", ins=[], outs=[],
                                         engine=inst.engine)
                    nop.sync_info = mybir.SyncInfo(on_wait=[w], on_update=[])
                    out.append(nop)
                inst.sync_info = mybir.SyncInfo(
                    on_wait=keep,
                    on_update=list(si.on_update) if si.on_update else [])
                changed = True
            out.append(inst)
        if changed:
            blk.instructions = out
    return n


def _aux_arrays(Wq, Wk, Wv, Wo, bo):
    """Host-side constant prep: pad head pairs to 64-aligned rows."""
    def pad_pairs_cols(W):
        # W [c, 320] -> [c, 4, 128]: cols 0:40 = head 2p, 64:104 = head 2p+1
        c = W.shape[0]
        out = np.zeros((c, NPAIR, 128), dtype=np.float32)
        for p in range(NPAIR):
            out[:, p, 0:40] = W[:, 80 * p: 80 * p + 40]
            out[:, p, 64:104] = W[:, 80 * p + 40: 80 * p + 80]
        return out.reshape(c, NPAIR * 128)

    wq_pad = pad_pairs_cols(Wq)                      # [320, 512]
    wk_pad = pad_pairs_cols(Wk)                      # [768, 512]
    # Wo rows padded to pair layout: [4, 128, 320]
    wo_pad = np.zeros((NPAIR, 128, QD), dtype=np.float32)
    for p in range(NPAIR):
        wo_pad[p, 0:40] = Wo[80 * p: 80 * p + 40]
        wo_pad[p, 64:104] = Wo[80 * p + 40: 80 * p + 80]
    # E: [8, 4*128] broadcast map R_h -> pair rows
    e_mat = np.zeros((H, NPAIR * 128), dtype=np.float32)
    for p in range(NPAIR):
        e_mat[2 * p, 128 * p: 128 * p + 40] = 1.0
        e_mat[2 * p + 1, 128 * p + 64: 128 * p + 104] = 1.0
    # selector columns for Zs accumulation: [77, 8], col h = ones
    zcol = np.zeros((M, H, H), dtype=np.float32)
    for h in range(H):
        zcol[:, h, h] = 1.0
    zcol = zcol.reshape(M, H * H)
    ident = np.eye(128, dtype=np.float32)
    return {
        "aux_wq": wq_pad, "aux_wk": wk_pad, "aux_wo": wo_pad.reshape(NPAIR * 128, QD),
        "aux_e": e_mat, "aux_z": zcol, "aux_id": ident,
        "aux_bo": bo.reshape(1, QD).astype(np.float32),
    }


def _build(nc, with_bias):
    import concourse.mybir as mybir
    from concourse.tile import TileContext

    R = mybir.dt.float32r
    F = mybir.dt.float32
    AF = mybir.ActivationFunctionType

    x_d = nc.dram_tensor("x", [B_LOC, N, QD], F, kind="ExternalInput")
    c_d = nc.dram_tensor("context", [B_LOC, M, CD], F, kind="ExternalInput")
    wq_d = nc.dram_tensor("aux_wq", [QD, 512], F, kind="ExternalInput")
    wk_d = nc.dram_tensor("aux_wk", [CD, 512], F, kind="ExternalInput")
    wv_d = nc.dram_tensor("Wv", [CD, ID], F, kind="ExternalInput")
    wo_d = nc.dram_tensor("aux_wo", [512, QD], F, kind="ExternalInput")
    e_d = nc.dram_tensor("aux_e", [H, 512], F, kind="ExternalInput")
    z_d = nc.dram_tensor("aux_z", [M, H * H], F, kind="ExternalInput")
    id_d = nc.dram_tensor("aux_id", [128, 128], F, kind="ExternalInput")
    bo_d = nc.dram_tensor("aux_bo", [1, QD], F, kind="ExternalInput")
    out_d = nc.dram_tensor("out", [B_LOC, N, QD], F, kind="ExternalOutput")

    CKT = [(0, 128), (128, 128), (256, 64)]            # QD k-tiles
    CKT6 = [(128 * i, 128) for i in range(6)]          # CD k-tiles

    with TileContext(nc) as tc:
        with tc.tile_pool(name="wpool", bufs=1) as wp, \
             tc.tile_pool(name="bpool", bufs=2) as bp, \
             tc.tile_pool(name="cpool", bufs=2) as cp, \
             tc.tile_pool(name="epool", bufs=10) as ep, \
             tc.tile_pool(name="ps", bufs=2, space="PSUM") as ps:

            # ---- per-core constants ----
            # identity + ctx first: they gate the first PE work (transposes)
            id_r = wp.tile([128, 128], R, name="id_r", tag="id_r")
            nc.sync.dma_start(id_r[:], id_d[:].bitcast(R))
            id_f = wp.tile([128, 128], F, name="id_f", tag="id_f")
            nc.sync.dma_start(id_f[:], id_d[:])
            ctx_tiles = []
            for b in range(B_LOC):
                t = bp.tile([M, CD], F, name=f"ctx{b}", tag="ctx")
                nc.sync.dma_start(t[:], c_d[b])
                ctx_tiles.append(t)
            wq_r = []
            for kt, (o, w) in enumerate(CKT):
                t = wp.tile([w, 512], R, name=f"wq{kt}", tag=f"wq{kt}")
                nc.sync.dma_start(t[:], wq_d[o:o + w, :].bitcast(R))
                wq_r.append(t)
            wk_r = []
            wv_r = []
            for kt, (o, w) in enumerate(CKT6):
                t = wp.tile([w, 512], R, name=f"wk{kt}", tag=f"wk{kt}")
                nc.sync.dma_start(t[:], wk_d[o:o + w, :].bitcast(R))
                wk_r.append(t)
                t2 = wp.tile([w, ID], R, name=f"wv{kt}", tag=f"wv{kt}")
                nc.sync.dma_start(t2[:], wv_d[o:o + w, :].bitcast(R))
                wv_r.append(t2)
            wo_r = []
            for p in range(NPAIR):
                t = wp.tile([128, QD], R, name=f"wo{p}", tag=f"wo{p}")
                nc.sync.dma_start(t[:], wo_d[128 * p:128 * p + 128, :].bitcast(R))
                wo_r.append(t)
            e_r = wp.tile([H, 512], R, name="e_r", tag="e_r")
            nc.sync.dma_start(e_r[:], e_d[:].bitcast(R))
            z_r = wp.tile([M, H * H], R, name="z_r", tag="z_r")
            nc.sync.dma_start(z_r[:], z_d[:].bitcast(R))
            if with_bias:
                bo_r = wp.tile([1, QD], R, name="bo_r", tag="bo_r")
                nc.sync.dma_start(bo_r[:], bo_d[:].bitcast(R))
                ones_r = wp.tile([1, 128], R, name="ones_r", tag="ones_r")
                nc.vector.memset(ones_r[:].bitcast(F), 1.0)

            for b in range(B_LOC):
                # ---- per-batch: ctx^T, k^T pairs, v ----
                ctx_f = ctx_tiles[b]
                ctxT_r = []
                for s, (o, w) in enumerate(CKT6):
                    pt = ps.tile([w, M], F, name=f"ctT{s}", tag="xtq")
                    nc.tensor.transpose(pt[:], ctx_f[:, o:o + w], id_f[:M, :M])
                    st = bp.tile([w, 80], R, name=f"ctxT{s}", tag=f"ctxT{s}")
                    nc.vector.memset(st[:, M:80].bitcast(F), 0.0)
                    nc.vector.tensor_copy(st[:, 0:M], pt[:])
                    ctxT_r.append(st)

                kT_r = []
                for p in range(NPAIR):
                    pt = ps.tile([128, 80], F, name=f"kps{p}", tag="xtq")
                    for kt in range(6):
                        nc.tensor.matmul(pt[:], wk_r[kt][:, 128 * p:128 * (p + 1)],
                                         ctxT_r[kt][:], start=(kt == 0), stop=(kt == 5))
                    st = bp.tile([128, 80], R, name=f"kT{p}", tag=f"kT{p}")
                    nc.vector.tensor_copy(st[:], pt[:])
                    kT_r.append(st)

                v_ps = ps.tile([M, ID], F, name="v_ps", tag="xtq")
                for kt in range(6):
                    nc.tensor.matmul(v_ps[:], ctxT_r[kt][:, 0:M], wv_r[kt][:],
                                     start=(kt == 0), stop=(kt == 5))
                v_r = bp.tile([M, H, 64], R, name="v_r", tag="v_r")
                nc.vector.memset(v_r[:, :, 40:64].bitcast(F), 0.0)
                nc.vector.tensor_copy(
                    v_r[:, :, 0:40],
                    v_ps[:].rearrange("p (h d) -> p h d", h=H)[:, :, 0:40])

                for c in range(NCHUNK):
                    t0 = CHUNK * c
                    # ---- x load + transpose ----
                    x_sb = cp.tile([128, 4, QD], R, name="x_sb", tag="x_sb")
                    nc.sync.dma_start(
                        x_sb[:],
                        x_d[b, t0:t0 + CHUNK, :].rearrange(
                            "(i p) cc -> p i cc", p=128).bitcast(R))
                    xT_r = []
                    for s, (o, w) in enumerate(CKT):
                        pt = ps.tile([w, CHUNK], F, name=f"xTps{s}", tag="xtq")
                        for i in range(4):
                            nc.tensor.transpose(
                                pt[:, 128 * i:128 * (i + 1)].bitcast(R),
                                x_sb[:, i, o:o + w], id_r[:])
                        st = cp.tile([w, CHUNK], R, name=f"xT{s}", tag=f"xT{s}")
                        nc.vector.tensor_copy(st[:], pt[:])
                        xT_r.append(st)

                    # ---- q^T head pairs ----
                    qT_r = []
                    for p in range(NPAIR):
                        pt = ps.tile([128, CHUNK], F, name=f"qps{p}", tag="xtq")
                        for kt in range(3):
                            nc.tensor.matmul(pt[:], wq_r[kt][:, 128 * p:128 * (p + 1)],
                                             xT_r[kt][:], start=(kt == 0), stop=(kt == 2))
                        st = cp.tile([128, CHUNK], R, name=f"qT{p}", tag=f"qT{p}")
                        nc.vector.tensor_copy(st[:], pt[:])
                        qT_r.append(st)

                    # ---- scores + exp per head ----
                    expS = []
                    for h in range(H):
                        p, j = h // 2, h % 2
                        base = 64 * j
                        spt = ps.tile([M, CHUNK], F, name=f"s{h}", tag="S")
                        nc.tensor.matmul(
                            spt[:], kT_r[p][base:base + DH, 0:M],
                            qT_r[p][base:base + DH, :], start=True, stop=True)
                        et = ep.tile([M, CHUNK], R, name=f"expS{h}", tag="expS")
                        nc.scalar.activation(et[:], spt[:], AF.Exp, scale=SCALE)
                        expS.append(et)

                    # ---- Zs = per-head sums via selector columns ----
                    zs_ps = ps.tile([H, CHUNK], F, name="zs_ps", tag="fz")
                    for h in range(H):
                        nc.tensor.matmul(zs_ps[:], z_r[:, H * h:H * (h + 1)],
                                         expS[h][:], start=(h == 0), stop=(h == H - 1))
                    lnz = cp.tile([H, CHUNK], F, name="lnz", tag="lnz")
                    nc.scalar.activation(lnz[:], zs_ps[:], AF.Ln)
                    rs_r = cp.tile([H, CHUNK], R, name="rs_r", tag="rs_r")
                    nc.scalar.activation(rs_r[:], lnz[:], AF.Exp, scale=-1.0)

                    # ---- O pairs (col-tiled) + normalize on copy ----
                    o_sb = []
                    for p in range(NPAIR):
                        opa = ps.tile([64, CHUNK], F, name=f"opa{p}", tag="O")
                        nc.tensor.matmul(opa[:], v_r[:, 2 * p, :],
                                         expS[2 * p][:], start=True, stop=True)
                        opb = ps.tile([64, CHUNK], F, name=f"opb{p}", tag="O")
                        nc.tensor.matmul(opb[:], v_r[:, 2 * p + 1, :],
                                         expS[2 * p + 1][:], start=True, stop=True)
                        rb_ps = ps.tile([128, CHUNK], F, name=f"rb{p}", tag="fz")
                        nc.tensor.matmul(rb_ps[:], e_r[:, 128 * p:128 * (p + 1)],
                                         rs_r[:], start=True, stop=True)
                        rb_sb = cp.tile([128, CHUNK], F, name=f"rbs{p}", tag="rb_sb")
                        nc.scalar.copy(rb_sb[:], rb_ps[:])
                        ot = cp.tile([128, CHUNK], R, name=f"osb{p}", tag=f"osb{p}")
                        nc.vector.tensor_tensor(out=ot[0:64, :], in0=opa[:],
                                                in1=rb_sb[0:64, :],
                                                op=mybir.AluOpType.mult)
                        nc.vector.tensor_tensor(out=ot[64:128, :], in0=opb[:],
                                                in1=rb_sb[64:128, :],
                                                op=mybir.AluOpType.mult)
                        o_sb.append(ot)

                    # ---- final projection + store ----
                    for i in range(4):
                        fp = ps.tile([128, QD], F, name=f"fin{i}", tag="fz")
                        for p in range(NPAIR):
                            nc.tensor.matmul(fp[:], o_sb[p][:, 128 * i:128 * (i + 1)],
                                             wo_r[p][:], start=(p == 0),
                                             stop=(p == NPAIR - 1 and not with_bias))
                        if with_bias:
                            nc.tensor.matmul(fp[:], ones_r[:], bo_r[:],
                                             start=False, stop=True)
                        ft = cp.tile([128, QD], F, name=f"fout{i}", tag=f"fout{i}")
                        nc.vector.tensor_copy(ft[:], fp[:])
                        nc.sync.dma_start(
                            out_d[b, t0 + 128 * i: t0 + 128 * (i + 1), :], ft[:])

    _legalize_sync_waits(nc, mybir)
    return nc


def _get_module(with_bias):
    key = ("mod", with_bias)
    if key not in _cache:
        import concourse.bass as bass
        _cache[key] = _build(bass.Bass(), with_bias)
    return _cache[key]


def kernel(x, context, Wq, Wk, Wv, Wo, bo):
    import os
    import sys
    if os.environ.get("JAX_PLATFORMS") == "cpu" and "jax" not in sys.modules:
        del os.environ["JAX_PLATFORMS"]
    from concourse.bass_utils import run_bass_kernel_spmd

    x = np.ascontiguousarray(x, dtype=np.float32)
    context = np.ascontiguousarray(context, dtype=np.float32)
    with_bias = bool(np.any(bo))
    aux = _aux_arrays(np.asarray(Wq, dtype=np.float32),
                      np.asarray(Wk, dtype=np.float32),
                      np.asarray(Wv, dtype=np.float32),
                      np.asarray(Wo, dtype=np.float32),
                      np.asarray(bo, dtype=np.float32))
    nc = _get_module(with_bias)

    in_maps = []
    for core in range(N_CORES):
        sl = slice(B_LOC * core, B_LOC * (core + 1))
        m = {"x": np.ascontiguousarray(x[sl]),
             "context": np.ascontiguousarray(context[sl]),
             "Wv": np.ascontiguousarray(np.asarray(Wv, dtype=np.float32))}
        m.update(aux)
        in_maps.append(m)

    res = run_bass_kernel_spmd(nc, in_maps, core_ids=list(range(N_CORES)))
    return np.concatenate([r["out"] for r in res.results], axis=0)
